# revision 1
# baseline (speedup 1.0000x reference)
"""GATv2 + softmax head for nn_GATModel_Softmax_4535485465120 on 8 trn2 NeuronCores.

Strategy (graph/data parallel by dst node, fully local — no collectives):
  - Nodes are partitioned into 8 ranges of 1000 dst nodes (one per core).
  - Each core receives (host-prepped, bf16, pre-transposed/gathered):
      * x.T columns for every edge slot's src node (edges sorted by dst,
        grouped into per-dst-block slots padded to a uniform tile count)
      * x.T columns for its 1000 dst nodes
      * weight matrices with |att| folded into Wl/Wr columns and, per head,
        columns permuted so positive-att features come first; Wf rows carry
        the inverse permutation, the 1/(3*|att|) un-scaling and the head-mean.
  - Device pipeline per core:
      phase 0: XR' = xdst @ Wr'                    -> HBM   [1024, 3072]
      phase 1: per 128-edge tile: XL'[src] via matmul + XR'[dst] added via a
               0/1 selection matmul into the same PSUM; U = that sum (bf16);
               leaky-relu + sign-segmented accumulation gives edge scores;
               a = exp(score). U tiles stored to HBM, a kept in SBUF.
      phase 2: per dst block: denom = segsum(a) via selection matmul;
               alpha = a * recip(denom)[dst] via selection matmul expansion;
               agg = sum_e alpha * U  (selection matmul), minus XR' (since
               sum(alpha)=1); transpose; logits = agg^T-chunks @ Wf_stack
               (+folded bias row); row softmax -> output.

kernel(**inputs) takes FULL inputs, shards internally, returns FULL [8000,460] f32.
"""

import numpy as np
import ml_dtypes

BF16 = ml_dtypes.bfloat16

# Problem constants (hardcoded per spec)
N = 8000
DIN = 1024
H = 3
C = 1024
HC = H * C          # 3072
NCLS = 460
NCLS_P = 512
NEG_SLOPE = 0.2
NCORES = 8
ND = N // NCORES    # 1000 dst nodes per core
NDP = 1024          # padded dst count per core
DB = NDP // 128     # 8 dst blocks per core
P = 128
KC = DIN // P       # 8 contraction chunks
NB = 3              # n-chunks of 1024 in HC
HF = 2              # 512-wide matmul halves per 1024 chunk


def _prep(x, edge_index, Wl, bl, Wr, br, att, bias, Wf, bf):
    """Host-side preprocessing -> per-core input maps + static dims."""
    x = np.asarray(x, np.float32)
    ei = np.asarray(edge_index).astype(np.int64)
    Wl = np.asarray(Wl, np.float32)
    Wr = np.asarray(Wr, np.float32)
    bl = np.asarray(bl, np.float32)
    br = np.asarray(br, np.float32)
    att = np.asarray(att, np.float32)
    bias = np.asarray(bias, np.float32)
    Wf = np.asarray(Wf, np.float32)
    bf = np.asarray(bf, np.float32)

    assert np.all(bl == 0) and np.all(br == 0), \
        "nonzero bl/br not supported by this kernel build"

    loops = np.arange(N, dtype=np.int64)
    src_all = np.concatenate([ei[:, 0], loops])
    dst_all = np.concatenate([ei[:, 1], loops])

    # att folding: per head, column scale |att| and permutation pos-first
    absatt = np.abs(att)                       # [H, C]
    perm = np.zeros((H, C), np.int64)          # perm[h, newc] = origc
    npos = np.zeros(H, np.int64)
    for h in range(H):
        pos = np.nonzero(att[h] > 0)[0]
        neg = np.nonzero(att[h] <= 0)[0]
        perm[h] = np.concatenate([pos, neg])
        npos[h] = len(pos)

    # scaled/permuted projection weights  [DIN, HC]
    Wl_s = np.zeros((DIN, HC), np.float32)
    Wr_s = np.zeros((DIN, HC), np.float32)
    for h in range(H):
        sc = absatt[h, perm[h]]                # [C]
        Wl_s[:, h * C:(h + 1) * C] = Wl[:, h * C + perm[h]] * sc
        Wr_s[:, h * C:(h + 1) * C] = Wr[:, h * C + perm[h]] * sc

    # final fc stack: logits = sum_h (agg'_h / (3*|att_h|)) @ Wf  (+ bias@Wf + bf)
    Wfs = np.zeros((HC, NCLS_P), np.float32)
    for h in range(H):
        sc = 1.0 / (3.0 * np.maximum(absatt[h, perm[h]], 1e-30))
        Wfs[h * C:(h + 1) * C, :NCLS] = Wf[perm[h]] * sc[:, None]
    bf2 = np.full((1, NCLS_P), -1e30, np.float32)
    bf2[0, :NCLS] = bias @ Wf + bf

    xT_bf = np.ascontiguousarray(x.T).astype(BF16)   # [DIN, N]

    # per-core edge grouping: edges (incl. self loops) by dst block
    cores = []
    tmax = 1
    for k in range(NCORES):
        lo, hi = k * ND, (k + 1) * ND
        m = (dst_all >= lo) & (dst_all < hi)
        s_k = src_all[m]
        dl_k = (dst_all[m] - lo).astype(np.int64)
        order = np.argsort(dl_k, kind="stable")
        s_k, dl_k = s_k[order], dl_k[order]
        blocks = []
        for db in range(DB):
            bm = (dl_k >= db * 128) & (dl_k < (db + 1) * 128)
            blocks.append((s_k[bm], dl_k[bm]))
            tmax = max(tmax, (len(blocks[-1][0]) + 127) // 128)
        cores.append(blocks)

    T_BLK = tmax
    E1T = DB * T_BLK
    E1P = E1T * 128

    iotaF = np.tile(np.arange(128, dtype=np.float32)[None, :], (128, 1))
    iotaP = np.tile(np.arange(128, dtype=np.float32)[:, None], (1, 128))
    ones1 = np.ones((1, 128), BF16)
    # unique srcs per core -> common padded tile count
    uniq = []
    for k in range(NCORES):
        srcs = np.concatenate([cores[k][db][0] for db in range(DB)])
        uniq.append(np.unique(srcs))
    UT = max((len(u) + 127) // 128 for u in uniq)
    UP = UT * 128
    in_maps = []
    for k in range(NCORES):
        srcslot = np.zeros(E1P, np.int64)
        real = np.zeros(E1P, bool)
        dstloc = np.full(E1P, -1.0, np.float32)
        for db in range(DB):
            s_k, dl_k = cores[k][db]
            base = db * T_BLK * 128
            srcslot[base:base + len(s_k)] = s_k
            real[base:base + len(s_k)] = True
            dstloc[base:base + len(s_k)] = dl_k.astype(np.float32)
        u = uniq[k]
        xuT = np.zeros((DIN, UP), BF16)
        xuT[:, :len(u)] = xT_bf[:, u]
        srcloc = np.zeros((E1P, 1), np.int32)
        srcloc[real, 0] = np.searchsorted(u, srcslot[real]).astype(np.int32)
        xdstT = np.zeros((DIN, NDP), BF16)
        xdstT[:, :ND] = xT_bf[:, k * ND:(k + 1) * ND]
        dst_col = np.ascontiguousarray(dstloc.reshape(E1T, 128).T)   # [128, E1T]
        dst_row = np.tile(dstloc[None, :], (128, 1))                 # [128, E1P]
        in_maps.append({
            "xuT": xuT,
            "srcloc": srcloc,
            "xdstT": xdstT,
            "wl": Wl_s.astype(BF16),
            "wr": Wr_s.astype(BF16),
            "wfs": Wfs.astype(BF16),
            "bf2": bf2.astype(BF16),
            "dstcp": dst_col,
            "dstrow": dst_row,
            "iotaF": iotaF,
            "iotaP": iotaP,
            "ones1": ones1,
        })
    dims = {"T_BLK": T_BLK, "E1T": E1T, "E1P": E1P, "UT": UT,
            "npos": [int(v) for v in npos]}
    return in_maps, dims


def _build(dims, use_act_lrelu=False):
    """Trace the Bass/Tile program (identical for all cores).

    use_act_lrelu: leaky-relu+accum on ScalarE (HW path; CoreSim lacks Lrelu).
    """
    import concourse.bass as bass
    import concourse.mybir as mybir
    import concourse.tile as tile
    from concourse import bacc

    T_BLK, E1T, E1P = dims["T_BLK"], dims["E1T"], dims["E1P"]
    UT = dims["UT"]
    npos = dims["npos"]
    fp32 = mybir.dt.float32
    bf16 = mybir.dt.bfloat16
    AT = mybir.AluOpType
    AF = mybir.ActivationFunctionType

    nc = bacc.Bacc("TRN2", target_bir_lowering=False, debug=False)

    with tile.TileContext(nc) as tc:
        with tc.tile_pool(name="dram", bufs=1, space="DRAM") as dram:
            d_xuT = dram.tile([DIN, UT * 128], bf16, kind="ExternalInput", name="xuT", uniquify=False)
            d_srcloc = dram.tile([E1P, 1], mybir.dt.int32, kind="ExternalInput", name="srcloc", uniquify=False)
            d_xdstT = dram.tile([DIN, NDP], bf16, kind="ExternalInput", name="xdstT", uniquify=False)
            d_wl = dram.tile([DIN, HC], bf16, kind="ExternalInput", name="wl", uniquify=False)
            d_wr = dram.tile([DIN, HC], bf16, kind="ExternalInput", name="wr", uniquify=False)
            d_wfs = dram.tile([HC, NCLS_P], bf16, kind="ExternalInput", name="wfs", uniquify=False)
            d_bf2 = dram.tile([1, NCLS_P], bf16, kind="ExternalInput", name="bf2", uniquify=False)
            d_dstcp = dram.tile([128, E1T], fp32, kind="ExternalInput", name="dstcp", uniquify=False)
            d_dstrow = dram.tile([128, E1P], fp32, kind="ExternalInput", name="dstrow", uniquify=False)
            d_iotaF = dram.tile([128, 128], fp32, kind="ExternalInput", name="iotaF", uniquify=False)
            d_iotaP = dram.tile([128, 128], fp32, kind="ExternalInput", name="iotaP", uniquify=False)
            d_ones1 = dram.tile([1, 128], bf16, kind="ExternalInput", name="ones1", uniquify=False)
            d_out = dram.tile([NDP, NCLS_P], fp32, kind="ExternalOutput", name="out", uniquify=False)
            d_xrd = dram.tile([NDP, HC], bf16, name="xrd_i")
            d_xlu = dram.tile([UT * 128, HC], bf16, name="xlu_i")
            d_xrdT = dram.tile([HC, NDP], bf16, name="xrdT_i")
            d_usum = dram.tile([E1P, HC], bf16, name="usum_i")

            with tc.tile_pool(name="gsb", bufs=1) as gsb:
                # resident tensors
                wmat = None  # allocated from wpool below
                xdst_r = gsb.tile([128, KC, NDP], bf16, tag="sh24", bufs=1, name="xdst_r")
                nc.sync.dma_start(out=xdst_r[:], in_=d_xdstT[:].rearrange("(kc p) n -> p kc n", p=128))
                dstcp = gsb.tile([128, E1T], fp32, name="dstcp_r")
                nc.sync.dma_start(out=dstcp[:], in_=d_dstcp[:])
                iotaF = gsb.tile([128, 128], fp32, name="iotaF_r")
                nc.sync.dma_start(out=iotaF[:], in_=d_iotaF[:])
                iotaP = gsb.tile([128, 128], fp32, name="iotaP_r")
                nc.sync.dma_start(out=iotaP[:], in_=d_iotaP[:])
                ones1 = gsb.tile([1, 128], bf16, name="ones1_r")
                nc.sync.dma_start(out=ones1[:], in_=d_ones1[:])
                bf2 = gsb.tile([1, NCLS_P], bf16, name="bf2_r")
                nc.sync.dma_start(out=bf2[:], in_=d_bf2[:])
                a_all = gsb.tile([128, E1T * H], fp32, name="a_all_r")
                denr = gsb.tile([128, DB * H], fp32, name="denr_r")

                # ---------------- phase 0: XR' projection ----------------
                with (
                    tc.tile_pool(name="p01", bufs=1, space="PSUM") as ps1,
                    tc.tile_pool(name="sb0", bufs=2) as sb0,
                ):
                    with tc.tile_pool(name="wpool", bufs=1) as wpool:
                      wmat = wpool.tile([128, KC, HC], bf16, tag="wmat", bufs=1, name="wmat_r")
                      nc.sync.dma_start(out=wmat[:], in_=d_wr[:].rearrange("(kc p) n -> p kc n", p=128))
                      for dc in range(DB):
                          xr_sb = sb0.tile([128, HC], bf16, tag="xr_sb")
                          for nb in range(NB):
                              pp = ps1.tile([128, C], fp32, tag="pp", bufs=2, name=f"pp{nb}_c{dc}")
                              for kc in range(KC):
                                  for hf in range(HF):
                                      nc.tensor.matmul(
                                          pp[:, hf * 512:(hf + 1) * 512],
                                          xdst_r[:, kc, dc * 128:(dc + 1) * 128],
                                          wmat[:, kc, nb * C + hf * 512:nb * C + (hf + 1) * 512],
                                          start=(kc == 0), stop=(kc == KC - 1))
                              nc.vector.tensor_copy(out=xr_sb[:, nb * C:(nb + 1) * C], in_=pp[:])
                          nc.sync.dma_start(out=d_xrd[dc * 128:(dc + 1) * 128, :], in_=xr_sb[:])

                      # XR'^T: lhsT = Wr' chunks, rhs = xdst chunks -> [3072, 1024]
                      for ncc in range(HC // 128):
                          xrT_sb = sb0.tile([128, NDP], bf16, tag="xrT_sb")
                          pxt = ps1.tile([128, NDP], fp32, tag="pp", bufs=2, name=f"pxt_{ncc}")
                          for kc in range(KC):
                              for hf in range(HF):
                                  nc.tensor.matmul(
                                      pxt[:, hf * 512:(hf + 1) * 512],
                                      wmat[:, kc, ncc * 128:(ncc + 1) * 128],
                                      xdst_r[:, kc, hf * 512:(hf + 1) * 512],
                                      start=(kc == 0), stop=(kc == KC - 1))
                          nc.vector.tensor_copy(out=xrT_sb[:], in_=pxt[:])
                          nc.sync.dma_start(out=d_xrdT[ncc * 128:(ncc + 1) * 128, :], in_=xrT_sb[:])

                      # swap weights to Wl for phase 1 (same SBUF slot, WAR-ordered)
                      wmat2 = wpool.tile([128, KC, HC], bf16, tag="wmat", bufs=1, name="wmat_r2")
                      nc.sync.dma_start(out=wmat2[:], in_=d_wl[:].rearrange("(kc p) n -> p kc n", p=128))

                      # ---------------- phase 0.5: XLu' projection of unique srcs ----------------
                      for ut in range(UT):
                          xu_sb = sb0.tile([128, KC, 128], bf16, tag="xu_sb")
                          nc.sync.dma_start(
                              out=xu_sb[:],
                              in_=d_xuT[:, ut * 128:(ut + 1) * 128]
                                  .rearrange("(kc p) e -> p kc e", p=128))
                          xl_sb = sb0.tile([128, HC], bf16, tag="xr_sb")
                          for nb in range(NB):
                              pp = ps1.tile([128, C], fp32, tag="pp", bufs=2, name=f"ppu{nb}_{ut}")
                              for kc in range(KC):
                                  for hf in range(HF):
                                      nc.tensor.matmul(
                                          pp[:, hf * 512:(hf + 1) * 512],
                                          xu_sb[:, kc, :],
                                          wmat2[:, kc, nb * C + hf * 512:nb * C + (hf + 1) * 512],
                                          start=(kc == 0), stop=(kc == KC - 1))
                              nc.scalar.activation(out=xl_sb[:, nb * C:(nb + 1) * C],
                                                   in_=pp[:], func=AF.Copy)
                          nc.sync.dma_start(out=d_xlu[ut * 128:(ut + 1) * 128, :], in_=xl_sb[:])

                    # wfs resident (reuses the xdst slot; phase 0 is done with it)
                    wfs_r = gsb.tile([128, HC // 128, NCLS_P], bf16, tag="sh24", bufs=1, name="wfs_r")
                    nc.sync.dma_start(out=wfs_r[:], in_=d_wfs[:].rearrange("(cc p) n -> p cc n", p=128))

                    # ------- fused phases 1+2: per dst block -------
                    with (
                        tc.tile_pool(name="sb1", bufs=3) as sb1,
                        tc.tile_pool(name="sbe", bufs=T_BLK + 4) as sbe,
                        tc.tile_pool(name="p2", bufs=1, space="PSUM") as ps2,
                        tc.tile_pool(name="sb2", bufs=3) as sb2,
                    ):
                        for db in range(DB):
                            xrdb = sb0.tile([128, HC], bf16, tag="xrdb", bufs=3)
                            nc.sync.dma_start(out=xrdb[:], in_=d_xrd[db * 128:(db + 1) * 128, :])
                            usums, esels, sals_all, drows = [], [], [], []
                            for t2 in range(T_BLK):
                                t = db * T_BLK + t2
                                sidx = sb1.tile([128, 1], mybir.dt.int32, tag="sidx")
                                nc.sync.dma_start(out=sidx[:], in_=d_srcloc[t * 128:(t + 1) * 128, :])
                                drow = sbe.tile([128, 128], fp32, tag="drow", name=f"drow_{db}_{t2}")
                                nc.sync.dma_start(out=drow[:], in_=d_dstrow[:, t * 128:(t + 1) * 128])
                                drows.append(drow)
                                eselw = sb1.tile([128, 128], bf16, tag="eselw")
                                nc.vector.scalar_tensor_tensor(
                                    out=eselw[:], in0=drow[:],
                                    scalar=float(-db * 128), in1=iotaP[:],
                                    op0=AT.add, op1=AT.is_equal)
                                usum = sbe.tile([128, HC], bf16, tag="usum", name=f"usum_{db}_{t2}")
                                eacc = sb1.tile([128, 2 * H], fp32, tag="eacc")
                                scr = sb1.tile([128, C], fp32, tag="scr", bufs=1)
                                for nb in range(NB):
                                    pp = ps1.tile([128, C], fp32, tag="pp", bufs=2, name=f"pp{nb}_t{t}")
                                    for hf in range(HF):
                                        nc.tensor.matmul(
                                            pp[:, hf * 512:(hf + 1) * 512], eselw[:],
                                            xrdb[:, nb * C + hf * 512:nb * C + (hf + 1) * 512],
                                            start=True, stop=True)
                                    nc.scalar.activation(out=usum[:, nb * C:(nb + 1) * C],
                                                         in_=pp[:], func=AF.Copy)
                                # gather XL'[src] rows, then add on DVE (DMA CCE add corrupts on HW)
                                xle = sb1.tile([128, HC], bf16, tag="xle", bufs=2)
                                nc.gpsimd.indirect_dma_start(
                                    out=xle[:], out_offset=None, in_=d_xlu[:],
                                    in_offset=bass.IndirectOffsetOnAxis(ap=sidx[:, :1], axis=0))
                                nc.vector.tensor_tensor(out=usum[:], in0=usum[:],
                                                        in1=xle[:], op=AT.add)
                                usums.append(usum)
                                # scores: per head, pos/neg leaky-relu accumulation
                                for h in range(H):
                                    np_h = npos[h]
                                    segs = [(h * C, np_h, 2 * h), (h * C + np_h, C - np_h, 2 * h + 1)]
                                    for off, ln, j in segs:
                                        if ln == 0:
                                            nc.vector.memset(eacc[:, j:j + 1], 0.0)
                                            continue
                                        if use_act_lrelu:
                                            nc.scalar.activation(
                                                out=scr[:, :ln], in_=usum[:, off:off + ln],
                                                func=AF.Lrelu, alpha=NEG_SLOPE,
                                                accum_out=eacc[:, j:j + 1])
                                        else:
                                            nc.vector.scalar_tensor_tensor(
                                                out=scr[:, :ln], in0=usum[:, off:off + ln],
                                                scalar=NEG_SLOPE, in1=usum[:, off:off + ln],
                                                op0=AT.mult, op1=AT.max,
                                                accum_out=eacc[:, j:j + 1])
                                esc = sb1.tile([128, H], fp32, tag="esc")
                                nc.vector.tensor_tensor(
                                    out=esc[:], in0=eacc[:, 0:2 * H:2],
                                    in1=eacc[:, 1:2 * H:2], op=AT.subtract)
                                nc.scalar.activation(
                                    out=a_all[:, t * H:(t + 1) * H], in_=esc[:], func=AF.Exp)
                            # pass A: denominators
                            pden = ps2.tile([128, H], fp32, tag="psmall", bufs=2, name=f"pden_{db}")
                            for t2 in range(T_BLK):
                                t = db * T_BLK + t2
                                ee = sbe.tile([128, 128], fp32, tag="esel_et", name=f"eet_{db}_{t2}")
                                nc.vector.scalar_tensor_tensor(
                                    out=ee[:], in0=dstcp[:, t:t + 1].to_broadcast([128, 128]),
                                    scalar=float(-db * 128), in1=iotaF[:],
                                    op0=AT.add, op1=AT.is_equal)
                                esels.append(ee)
                                nc.tensor.matmul(
                                    pden[:], ee[:], a_all[:, t * H:(t + 1) * H],
                                    start=(t2 == 0), stop=(t2 == T_BLK - 1))
                            dtmp = sb2.tile([128, H], fp32, tag="dtmp")
                            nc.vector.tensor_scalar_add(out=dtmp[:], in0=pden[:], scalar1=1e-16)
                            nc.vector.reciprocal(out=denr[:, db * H:(db + 1) * H], in_=dtmp[:])
                            # pass B: alpha and selection weights
                            for t2 in range(T_BLK):
                                t = db * T_BLK + t2
                                esde = sb2.tile([128, 128], fp32, tag="esde", bufs=4)
                                nc.vector.scalar_tensor_tensor(
                                    out=esde[:], in0=drows[t2][:],
                                    scalar=float(-db * 128), in1=iotaP[:],
                                    op0=AT.add, op1=AT.is_equal)
                                pad = ps2.tile([128, H], fp32, tag="psmall", bufs=2, name=f"pad_{db}_{t2}")
                                nc.tensor.matmul(pad[:], esde[:], denr[:, db * H:(db + 1) * H],
                                                 start=True, stop=True)
                                alpha = sb2.tile([128, H], fp32, tag="alpha")
                                nc.vector.tensor_tensor(out=alpha[:], in0=a_all[:, t * H:(t + 1) * H],
                                                        in1=pad[:], op=AT.mult)
                                hsal = []
                                for h in range(H):
                                    sal = sb2.tile([128, 128], bf16, tag=f"sal{h}",
                                                   bufs=T_BLK + 4, name=f"sal{h}_{db}_{t2}")
                                    nc.vector.tensor_tensor(
                                        out=sal[:], in0=esels[t2][:],
                                        in1=alpha[:, h:h + 1].to_broadcast([128, 128]),
                                        op=AT.mult)
                                    hsal.append(sal)
                                sals_all.append(hsal)
                            # transposed aggregation in two 3-bank halves + drain + fc accumulate
                            xrdTb = sb2.tile([128, HC // 128, 128], bf16, tag="xrdTb", bufs=2)
                            nc.sync.dma_start(
                                out=xrdTb[:],
                                in_=d_xrdT[:, db * 128:(db + 1) * 128]
                                    .rearrange("(cc p) d -> p cc d", p=128))
                            oaggT = sb2.tile([128, HC // 128, 128], bf16, tag="oaggT", bufs=2)
                            NTHIRD = HC // 128 // 3
                            for third in range(3):
                                pagg = ps2.tile([128, NTHIRD * 128], fp32, tag="pagghalf",
                                                bufs=1, name=f"pagg_{db}_{third}")
                                for j in range(NTHIRD):
                                    cc = third * NTHIRD + j
                                    h = cc // (HC // 128 // H)
                                    for t2 in range(T_BLK):
                                        nc.tensor.matmul(
                                            pagg[:, j * 128:(j + 1) * 128],
                                            usums[t2][:, cc * 128:(cc + 1) * 128],
                                            sals_all[t2][h][:],
                                            start=(t2 == 0), stop=(t2 == T_BLK - 1))
                                for q in range(NTHIRD // 4):
                                    base = third * NTHIRD + q * 4
                                    nc.vector.scalar_tensor_tensor(
                                        out=oaggT[:, base:base + 4, :],
                                        in0=pagg[:, q * 512:(q + 1) * 512], scalar=1.0,
                                        in1=xrdTb[:, base:base + 4, :],
                                        op0=AT.mult, op1=AT.subtract)
                            pfc = ps2.tile([128, NCLS_P], fp32, tag="psmall", bufs=2, name=f"pfc_{db}")
                            for cc in range(HC // 128):
                                nc.tensor.matmul(
                                    pfc[:], oaggT[:, cc, :], wfs_r[:, cc, :],
                                    start=(cc == 0), stop=False)
                            nc.tensor.matmul(pfc[:], ones1[:], bf2[:], start=False, stop=True)
                            # softmax
                            negmax = sb2.tile([128, 1], fp32, tag="negmax")
                            nc.vector.tensor_reduce(out=negmax[:], in_=pfc[:],
                                                    axis=mybir.AxisListType.X,
                                                    op=AT.max, negate=True)
                            pexp = sb2.tile([128, NCLS_P], fp32, tag="pexp", bufs=2)
                            nc.scalar.activation(out=pexp[:], in_=pfc[:], func=AF.Exp,
                                                 bias=negmax[:, 0:1], scale=1.0)
                            ssum = sb2.tile([128, 1], fp32, tag="ssum")
                            nc.vector.tensor_reduce(out=ssum[:], in_=pexp[:],
                                                    axis=mybir.AxisListType.X, op=AT.add)
                            rs = sb2.tile([128, 1], fp32, tag="rs")
                            nc.vector.reciprocal(out=rs[:], in_=ssum[:])
                            hout = sb2.tile([128, NCLS_P], fp32, tag="hout", bufs=2)
                            nc.vector.scalar_tensor_tensor(
                                out=hout[:], in0=pexp[:], scalar=rs[:, 0:1], in1=pexp[:],
                                op0=AT.mult, op1=AT.bypass)
                            nc.sync.dma_start(out=d_out[db * 128:(db + 1) * 128, :], in_=hout[:])

    nc.compile()
    return nc


def kernel(**inputs):
    out_full = np.zeros((N, NCLS), np.float32)
    in_maps, dims = _prep(
        inputs["x"], inputs["edge_index"], inputs["Wl"], inputs["bl"],
        inputs["Wr"], inputs["br"], inputs["att"], inputs["bias"],
        inputs["Wf"], inputs["bf"])
    nc = _build(dims)
    from concourse.bass_utils import run_bass_kernel_spmd
    res = run_bass_kernel_spmd(nc, in_maps, core_ids=list(range(NCORES)))
    for k in range(NCORES):
        out_full[k * ND:(k + 1) * ND, :] = res.results[k]["out"][:ND, :NCLS]
    return out_full

